# revision 33
# baseline (speedup 1.0000x reference)
"""ApproxCompressor Trainium2 kernel (8 NeuronCores, data parallel over batch).

Algorithm: the reference's FFT convolution with the truncated exponential
impulse response h[n] = (1-a) a^n is a one-pole IIR y[t] = a y[t-1] + (1-a) e[t]
minus a tail term a^16384 y[t-16384] that underflows to zero in float32 for
any alpha = sigmoid(randn).  On-device we therefore run an exact recursive
scan instead of an FFT.

Per core: 4 examples, processed as a 4-deep pipeline.  Each example's
L=131072 samples are laid out as [128 partitions x 1024], so every DMA is a
fully contiguous 512KB HBM transfer (strided patterns measured ~80GB/s vs
~300GB/s contiguous) and all 128 partitions scan in parallel (DVE
tensor_tensor_scan along the free dim, one independent recurrence per
partition-chunk).

Cross-chunk scan carries are fixed post-hoc: carry[p] (the true initial
state of chunk p) is linear in the per-chunk final values S, carry = M @ S
with M precomputed on host in f64.  Two tiny TensorE matmuls compute
carryT = S^T @ M^T ([1,128]) then the rank-1 update carry[p] * a^(i+1)
(a^(i+1) underflows past ~600 samples so nb*512 columns suffice).  The
scan writes its first 512 columns into PSUM so the correction matmul
accumulates onto them directly (start=False) - no extra vector op.

The quadratic-knee gain is refactored into per-partition-scalar ops:
    d    = ln(e^{-thr} * (y + eps))            (ACT, scale/bias fold)
    u    = clamp(d, -W, W)                     (DVE tensor_scalar)
    sqv  = (s*u + s*W)^2,  s = sqrt(-c/(4W))   (ACT square)
    dm   = (d - W) * (-c)                      (DVE tensor_scalar)
    comb = max(dm, 0) + sqv                    (DVE scalar_tensor_tensor)
    gain = exp(-comb)                          (ACT)
which equals exp(c*q(d)) of the reference knee exactly (the energy path
runs in bf16: its 0.4% relative error enters the log domain additively
as ~4e-3, far inside tolerance).  out_c = gain * x_c
is computed in place into the input tiles, which are then DMA'd out
contiguously.  Ln/Exp/Square all live in the natural_log_exp_and_others
ACT table set; get_activation_tables is narrowed during compile so the
set chooser picks it (avoids per-chunk table reloads).
"""

import numpy as np

N, C, L = 32, 2, 131072
NCORES = 8
NE = N // NCORES          # examples per core
P = 128                   # partitions = chunks per example
F = L // P                # 1024 samples per partition
BANK = 512                # psum bank width for the carry fix
EPS = 1e-5

_CACHE = {}


def _build(nb):
    import concourse.bass as bass
    import concourse.tile as tile
    from concourse import bacc, mybir

    f32 = mybir.dt.float32
    AF = mybir.ActivationFunctionType
    OP = mybir.AluOpType

    nc = bacc.Bacc("TRN2", target_bir_lowering=False, debug=False, num_devices=NCORES)

    x_h = nc.declare_dram_parameter("x", [NE, C, L], f32, isOutput=False)
    scal_h = nc.declare_dram_parameter("scal", [P, 16 * NE], f32, isOutput=False)
    bf16 = mybir.dt.bfloat16
    mmt_h = nc.declare_dram_parameter("mmt", [NE * P, P], bf16, isOutput=False)
    dec_h = nc.declare_dram_parameter("decay", [1, NE * nb * BANK], bf16, isOutput=False)
    out_h = nc.declare_dram_parameter("out", [NE, C, L], f32, isOutput=True)

    from contextlib import ExitStack

    with tile.TileContext(nc) as tc, ExitStack() as ctx:
        const = ctx.enter_context(tc.tile_pool(name="const", bufs=1))
        work = ctx.enter_context(tc.tile_pool(name="work", bufs=4))
        ypool = ctx.enter_context(tc.tile_pool(name="ypool", bufs=4))
        xpool = ctx.enter_context(tc.tile_pool(name="xpool", bufs=4))
        psum = ctx.enter_context(tc.tile_pool(name="psum", bufs=3, space="PSUM"))

        # first example's inputs lead the sync ring; scal (32KB) follows and
        # still lands well before the first square needs it
        x_e0 = [xpool.tile([P, F], f32, tag=f"x{c}", name=f"x{c}e0") for c in range(C)]
        nc.sync.dma_start(x_e0[0][:], x_h[:][0, 0].rearrange("(p i) -> p i", p=P))
        nc.scalar.dma_start(x_e0[1][:], x_h[:][0, 1].rearrange("(p i) -> p i", p=P))
        scal_t = const.tile([P, 16 * NE], f32)
        nc.sync.dma_start(scal_t[:], scal_h[:])
        mmt_t = [const.tile([P, P], bf16, name=f"mmt{e}") for e in range(NE)]
        for e in range(NE):
            nc.gpsimd.dma_start(mmt_t[e][:], mmt_h[:][e * P : (e + 1) * P, :])
        dec_t = const.tile([1, NE * nb * BANK], bf16, padded_shape=[P, NE * nb * BANK])
        nc.gpsimd.dma_start(dec_t[:], dec_h[:])

        def sc(e, j):
            return scal_t[:, 16 * e + j : 16 * e + j + 1]

        # tiny dummy activation: hoists the ACT table load off the critical
        # path (otherwise it fires only after the first x DMA lands)
        warm_t = const.tile([P, 1], f32)
        nc.scalar.activation(warm_t[:], scal_t[:, 0:1], AF.Exp, bias=0.0, scale=0.0)

        for e in range(NE):
            if e == 0:
                x0, x1 = x_e0
            else:
                x0 = xpool.tile([P, F], f32, tag="x0")
                x1 = xpool.tile([P, F], f32, tag="x1")
                nc.gpsimd.dma_start(x0[:], x_h[:][e, 0].rearrange("(p i) -> p i", p=P))
                nc.gpsimd.dma_start(x1[:], x_h[:][e, 1].rearrange("(p i) -> p i", p=P))

            # energy e[t] = ((1-a)/2) * (x0^2 + x1^2), scale folded into the squares
            sq0 = work.tile([P, F], bf16, tag="sq0")
            sq1 = work.tile([P, F], bf16, tag="sq1")
            nc.scalar.activation(sq0[:], x0[:], AF.Square, bias=0.0, scale=sc(e, 0))
            nc.scalar.activation(sq1[:], x1[:], AF.Square, bias=0.0, scale=sc(e, 0))
            e_t = work.tile([P, F], bf16, tag="e")
            nc.vector.tensor_tensor(e_t[:], sq0[:], sq1[:], op=OP.add)

            # local scans, one independent recurrence per partition-chunk.
            # The first BANK columns land in PSUM so the carry correction can
            # be accumulated onto them by the TensorE (start=False) for free.
            yp = psum.tile([P, BANK], f32, tag="yp")
            y_t = ypool.tile([P, F - BANK], f32, tag="y")
            ab = sc(e, 1).broadcast_to([P, BANK])
            nc.vector.tensor_tensor_scan(
                yp[:], ab, e_t[:, :BANK], 0.0, op0=OP.mult, op1=OP.add,
            )
            nc.vector.tensor_tensor_scan(
                y_t[:], sc(e, 1).broadcast_to([P, F - BANK]), e_t[:, BANK:],
                yp[:, BANK - 1 : BANK], op0=OP.mult, op1=OP.add,
            )

            # carry fix: carryT = S^T @ M^T, then y[:, :nb*512] += carry x decay
            s_bf = work.tile([P, 1], bf16, tag="s_bf")
            nc.vector.tensor_copy(s_bf[:], y_t[:, F - BANK - 1 : F - BANK])
            p1 = psum.tile([1, P], f32, tag="p1", bufs=2)
            nc.tensor.matmul(p1[:], s_bf[:], mmt_t[e][:], start=True, stop=True)
            carryT = work.tile([1, P], bf16, tag="carryT", padded_shape=[P, P])
            nc.scalar.copy(carryT[:], p1[:])
            off = e * nb * BANK
            nc.tensor.matmul(
                yp[:], carryT[:], dec_t[0:1, off : off + BANK],
                start=False, stop=True, skip_group_check=True,
            )
            for b in range(1, nb):
                pc = psum.tile([P, BANK], f32, tag="pc")
                nc.tensor.matmul(
                    pc[:], carryT[:],
                    dec_t[0:1, off + b * BANK : off + (b + 1) * BANK],
                    start=True, stop=True,
                )
                ysl = y_t[:, (b - 1) * BANK : b * BANK]
                nc.vector.tensor_add(ysl, ysl, pc[:])

            # knee gain
            d_t = work.tile([P, F], f32, tag="d")
            nc.scalar.activation(d_t[:, :BANK], yp[:], AF.Ln,
                                 bias=sc(e, 3), scale=sc(e, 2))
            nc.scalar.activation(d_t[:, BANK:], y_t[:], AF.Ln,
                                 bias=sc(e, 3), scale=sc(e, 2))
            u_t = work.tile([P, F], f32, tag="u")
            nc.vector.tensor_scalar(u_t[:], d_t[:], sc(e, 4), sc(e, 5),
                                    op0=OP.max, op1=OP.min)
            sqv_t = work.tile([P, F], f32, tag="sqv")
            nc.scalar.activation(sqv_t[:], u_t[:], AF.Square,
                                 bias=sc(e, 8), scale=sc(e, 7))
            dm_t = work.tile([P, F], f32, tag="dm")
            nc.vector.tensor_scalar(dm_t[:], d_t[:], sc(e, 5), sc(e, 6),
                                    op0=OP.subtract, op1=OP.mult)
            comb_t = work.tile([P, F], f32, tag="comb")
            nc.vector.scalar_tensor_tensor(comb_t[:], dm_t[:], 0.0, sqv_t[:],
                                           op0=OP.max, op1=OP.add)
            g_t = work.tile([P, F], f32, tag="g")
            nc.scalar.activation(g_t[:], comb_t[:], AF.Exp, bias=0.0, scale=-1.0)

            # gain application in place, then contiguous DMA out
            nc.vector.tensor_mul(x1[:], g_t[:], x1[:])
            out_eng = nc.scalar if e == NE - 1 else nc.sync
            out_eng.dma_start(out_h[:][e, 1].rearrange("(p i) -> p i", p=P), x1[:])
            nc.vector.tensor_mul(x0[:], g_t[:], x0[:])
            nc.sync.dma_start(out_h[:][e, 0].rearrange("(p i) -> p i", p=P), x0[:])

    # narrow the ACT table sets so Ln/Exp/Square resolve to the one set that
    # holds all three -> a single table load instead of per-chunk reloads
    import concourse.bacc as bacc_mod

    orig = bacc_mod.get_activation_tables
    strip = {AF.Ln, AF.Exp, AF.Square}

    def patched(arch):
        full = orig(arch)
        return {
            name: (set(fns) if name == "natural_log_exp_and_others"
                   else set(fns) - strip)
            for name, fns in full.items()
        }

    bacc_mod.get_activation_tables = patched
    try:
        nc.compile()
    finally:
        bacc_mod.get_activation_tables = orig
    return nc


def _host_consts(lt, lr, lk, za, nb):
    """Per-core constant tensors from the [NE] parameter vectors (f64 math)."""
    alpha = 1.0 / (1.0 + np.exp(-za))
    thr = lt - 6.0
    r = 1.0 + np.exp(lr)
    c = 1.0 / r - 1.0
    W = np.exp(lk) / 2.0

    cols = np.zeros((NE, 16))
    cols[:, 0] = np.sqrt((1.0 - alpha) / 2.0)     # s1: energy scale
    cols[:, 1] = alpha
    cols[:, 2] = np.exp(-thr)                     # lnscale
    cols[:, 3] = EPS * np.exp(-thr)               # lnbias
    cols[:, 4] = -W
    cols[:, 5] = W
    cols[:, 6] = -c
    cols[:, 7] = np.sqrt(-c / (4.0 * W))          # s
    cols[:, 8] = np.sqrt(-c / (4.0 * W)) * W      # s*W
    cols[:, 9] = c * W                            # prelu bias
    scal = np.tile(cols.reshape(1, NE * 16), (P, 1)).astype(np.float32)

    # carry matrix, transposed for the matmul: mmt[e][q, p] = A^(p-1-q), q < p
    A = alpha**F
    mmt = np.zeros((NE, P, P))
    qs = np.arange(P)
    for e in range(NE):
        for p in range(1, P):
            mmt[e, :p, p] = A[e] ** (p - 1 - qs[:p])
    import ml_dtypes
    mmt = mmt.reshape(NE * P, P).astype(ml_dtypes.bfloat16)

    dec = (alpha[:, None] ** np.arange(1, nb * BANK + 1)[None, :])
    import ml_dtypes
    dec = dec.astype(ml_dtypes.bfloat16)
    return {"scal": scal, "mmt": mmt, "decay": dec.reshape(1, NE * nb * BANK)}


def _pick_nb(za):
    alpha_max = float(1.0 / (1.0 + np.exp(-np.max(za))))
    alpha_max = min(max(alpha_max, 1e-6), 1.0 - 1e-9)
    need = np.log(1e-10) / np.log(alpha_max)
    return int(min(max(np.ceil(need / BANK), 1), F // BANK))


def _prep(inputs):
    x = np.ascontiguousarray(np.asarray(inputs["input_signals"], np.float32))
    lt = np.asarray(inputs["log_threshold"], np.float64).reshape(N)
    lr = np.asarray(inputs["log_ratio"], np.float64).reshape(N)
    lk = np.asarray(inputs["log_knee"], np.float64).reshape(N)
    za = np.asarray(inputs["z_alpha_pre"], np.float64).reshape(N)
    nb = _pick_nb(za)
    in_maps = []
    for i in range(NCORES):
        s = slice(i * NE, (i + 1) * NE)
        m = {"x": x[s]}
        m.update(_host_consts(lt[s], lr[s], lk[s], za[s], nb))
        in_maps.append(m)
    return nb, in_maps


def _get_nc(nb):
    if nb not in _CACHE:
        _CACHE[nb] = _build(nb)
    return _CACHE[nb]


def _run(inputs, trace=False):
    from concourse.bass_utils import run_bass_kernel_spmd

    nb, in_maps = _prep(inputs)
    nc = _get_nc(nb)
    res = run_bass_kernel_spmd(nc, in_maps, core_ids=list(range(NCORES)), trace=trace)
    out = np.concatenate([res.results[i]["out"] for i in range(NCORES)], axis=0)
    return out, res


def kernel(**inputs):
    out, _ = _run(inputs, trace=False)
    return out
